# revision 50
# baseline (speedup 1.0000x reference)
"""Trainium2 Bass kernel for a 3x3 VALID conv: x[64,256,256] * k[128,64,3,3] -> [128,254,254].

Strategy:
  - Shard output rows across 8 cores: core 0 takes 30 rows, cores 1-7 take 32
    rows each (30 + 7*32 = 254, no padded rows anywhere). One SPMD program;
    the 16th row-pair is guarded by `partition_id > 0`.
  - bf16 operands (PE rate identical to f32r in the cost model; halves DMA bytes).
  - 5 accumulated matmuls per pair of output rows (the chain lower bound for a
    576-lane contraction at K<=128), using two SBUF x layouts:
      xa: partitions 0..63 = x rows q,   64..127 = x rows q+1  (row-shifted dup)
          -> covers taps (kh=0,kw)+(kh=1,kw) for kw=0,1,2      (3 matmuls)
      xb: partitions 0..63 = x rows q+2, 64..127 = x rows q+2 col-shifted +1
          -> covers taps (2,0)+(2,1) in one K=128 matmul       (1 matmul)
          -> tap (2,2) as a K=64 matmul on xb's lower half     (1 matmul)
  - PE p-state warm-up: dummy matmuls on a memset scratch tile keep the PE busy
    from ~0.7us so matmuls hit full clock as soon as the ramp window allows.
  - DMA queue split: xa + weights on the ACT HWDGE queue, xb on SP, stores on
    SP (queues transfer concurrently in the cost model; no engine-compute is
    placed on ACT/SP so DMAs never wedge behind it).
  - PSUM evacuation (fp32 psum -> bf16 SBUF) on DVE.
  - Bias is added on the host after the gather (biases are zeros here; the add
    is exact fp32 either way).
"""

import os
import sys

import numpy as np

for _p in ("/opt/trn_rl_repo", "/root/.axon_site/_ro/trn_rl_repo"):
    if os.path.isdir(_p) and _p not in sys.path:
        sys.path.insert(0, _p)

from concourse import bass, mybir, tile  # noqa: E402
from concourse.bass_utils import run_bass_kernel_spmd  # noqa: E402

IN_C, H, W = 64, 256, 256
KS = 3
OUT_C = 128
OH, OW = H - KS + 1, W - KS + 1  # 254, 254
N_CORES = 8
RPC0 = 30         # output rows on core 0
RPC = 32          # output rows on cores 1..7  (30 + 7*32 = 254)

# 12 x 32-wide M=1 warm-up matmuls bridge from the tiny DVE memset (first
# warm-up at ~0.40us) to the first w/xa DMAs landing (~0.70us locally;
# j0+j1's weights ride one bundled SP DMA), keeping the PE busy-streak
# unbroken so full clock engages at wall-clock ~3.0us. Undershooting the
# handoff (N<=11) costs ~850ns (ramp restart) — resweep after any change
# to the early DMA order.
WARMUP_N = int(os.environ.get("CONV_WARMUP_N", "12"))
WARMUP_W = int(os.environ.get("CONV_WARMUP_W", "32"))
WARMUP_M = int(os.environ.get("CONV_WARMUP_M", "0"))

# Matmul dtype (kept for test.py compatibility; the kernel is bf16).
MM_DT = "bf16"

TRACE = False
LAST_RESULTS = None

_COMPILED = {}


def _np_bf16():
    import ml_dtypes

    return np.dtype(ml_dtypes.bfloat16)


def _chunk_bounds(rows):
    # [2, 4, 4, ...] — a small first chunk lets pair 0 start as early as
    # possible; later chunks amortize per-DMA overhead.
    bounds = [0, 2]
    while bounds[-1] < rows:
        bounds.append(min(bounds[-1] + 4, rows))
    return bounds


def _build_program(rows):
    """One SPMD program for all 8 cores: 16 row-pairs, with the last pair
    guarded by `partition_id > 0` — core 0 (rank 0) computes only 15 pairs
    (30 rows), cores 1..7 compute all 16 (32 rows); 30 + 7*32 = 254."""
    bf16 = mybir.dt.bfloat16
    f32 = mybir.dt.float32
    n_pairs = rows // 2
    nc = bass.Bass()

    xa_ext = nc.declare_dram_parameter("xa", [128, rows * W], bf16, isOutput=False)
    xb_ext = nc.declare_dram_parameter("xb", [128, rows * W], bf16, isOutput=False)
    w_ext = nc.declare_dram_parameter("wpack", [128, 5 * 128], bf16, isOutput=False)
    o_ext = nc.declare_dram_parameter("out", [128, rows * OW], bf16, isOutput=True)

    with tile.TileContext(nc) as tc:
        with (
            tc.tile_pool(name="wpool", bufs=1) as wpool,
            tc.tile_pool(name="xpool", bufs=1) as xpool,
            tc.tile_pool(name="pwarm", bufs=1, space="PSUM") as pwarm,
            tc.tile_pool(name="pspool", bufs=7, space="PSUM") as pspool,
            # bufs = n pairs: output tiles are never reused -> evacuations
            # only ever wait on their PSUM producer.
            tc.tile_pool(name="opool", bufs=n_pairs + 1) as opool,
        ):
            # PE p-state warm-up (costs nothing: PE is idle while loads
            # land). M=1 warm-ups only need a [128, 1+W] scratch: the tiny
            # DVE memset finishes sooner, so the PE busy-streak starts
            # earlier and the full-clock point moves up with it.
            warm = wpool.tile([128, 1 + WARMUP_W], bf16)
            nc.vector.memset(warm[:], 0.0)
            pw = pwarm.tile([1, WARMUP_W], f32)
            for _ in range(WARMUP_N):
                nc.tensor.matmul(
                    pw[:],
                    lhsT=warm[:, 0:1],
                    rhs=warm[:, 1 : 1 + WARMUP_W],
                    start=True,
                    stop=True,
                )
            for _ in range(WARMUP_M):
                nc.tensor.matmul(
                    pw[:, 0:8],
                    lhsT=warm[:, 0:1],
                    rhs=warm[:, 1:9],
                    start=True,
                    stop=True,
                )

            wt = wpool.tile([128, 5 * 128], bf16)
            # First DMAs on both queues land at ~700ns regardless of size
            # (fixed DGE costs dominate). Bundle w slots 0+1 into SP's first
            # DMA so j0 AND j1 are unblocked at ~800ns; w slots 2-4 follow
            # xa0 on ACT and land before j2 needs them.
            nc.sync.dma_start(out=wt[:, 0:256], in_=w_ext[:, 0:256])

            xat = xpool.tile([128, rows * W], bf16)
            xbt = xpool.tile([128, rows * W], bf16)
            bounds = _chunk_bounds(rows)
            for i, (q0, q1) in enumerate(zip(bounds[:-1], bounds[1:])):
                # First xa chunk rides the ACT queue in parallel with w0 on
                # SP (j0 needs both); everything else: xa on ACT, xb on SP.
                nc.scalar.dma_start(
                    out=xat[:, q0 * W : q1 * W], in_=xa_ext[:, q0 * W : q1 * W]
                )
                if i == 0:
                    nc.scalar.dma_start(out=wt[:, 256:], in_=w_ext[:, 256:])
                nc.sync.dma_start(
                    out=xbt[:, q0 * W : q1 * W], in_=xb_ext[:, q0 * W : q1 * W]
                )

            # Dummy activation AFTER the load dispatches: loads the ACT
            # function table off the critical path so the final pair's
            # evacuation can use the ACT engine.
            actwarm = wpool.tile([128, 1], bf16)
            nc.scalar.copy(actwarm[:], warm[:, 0:1])

            wv = wt[:].rearrange("p (s m) -> p s m", m=128)
            ov = o_ext.rearrange("p (r w) -> p r w", w=OW)
            xav = xat[:].rearrange("p (q w) -> p q w", w=W)
            xbv = xbt[:].rearrange("p (q w) -> p q w", w=W)

            pid = nc.partition_id()

            def _rows(r, nr, ps, evac, fine=False):
                """One PSUM accumulation group covering output rows r..r+nr.

                fine=True splits every matmul into quarter-size pieces: the
                p-state (mid vs full clock) is sampled at instruction issue,
                so fine pieces let the rate flip mid-group where a 508-wide
                matmul straddling the 3us full-clock point would pay mid
                rate for its whole duration.
                """
                psv = ps.rearrange("p (r w) -> p r w", w=OW)
                HW = OW // 2  # 127

                def mm(lhsT, rhs_view, base_c, start, stop):
                    if not fine:
                        nc.tensor.matmul(
                            psv[:, 0:nr, :],
                            lhsT=lhsT,
                            rhs=rhs_view[:, r : r + nr, base_c : base_c + OW],
                            start=start,
                            stop=stop,
                        )
                        return
                    n_piece = 2 * nr
                    pi = 0
                    for i in range(nr):
                        for c0 in (0, HW):
                            cw = OW - c0 if c0 == HW else HW
                            nc.tensor.matmul(
                                psv[:, i : i + 1, c0 : c0 + cw],
                                lhsT=lhsT,
                                rhs=rhs_view[
                                    :, r + i : r + i + 1,
                                    base_c + c0 : base_c + c0 + cw,
                                ],
                                start=start and pi == 0,
                                stop=stop and pi == n_piece - 1,
                            )
                            pi += 1

                for kw in range(3):
                    mm(wv[:, kw, :], xav, kw, kw == 0, False)
                mm(wv[:, 3, :], xbv, 0, False, False)
                mm(wv[0:64, 4, :], xbv[0:64], 2, False, True)
                so = opool.tile([128, nr * OW], bf16, name="so")
                evac(so[:], ps[:])
                nc.sync.dma_start(out=ov[:, r : r + nr, :], in_=so[:])

            def _pair(lp, tail=False, fine=False):
                r = 2 * lp
                ps = pspool.tile([128, 2 * OW], f32, name="ps")
                if tail:
                    # Exit critical path: ACT evac is slightly faster than DVE
                    # (table preloaded above).
                    _rows(r, 2, ps, lambda o, p: nc.scalar.copy(o, p))
                else:
                    _rows(r, 2, ps,
                          lambda o, p: nc.vector.tensor_scalar_add(o, p, 0.0),
                          fine=fine)

            # Pair 1 straddles the wall-clock 3us full-speed point: emit it
            # in quarter-size pieces so the clock-rate flip (sampled at
            # instruction issue) isn't delayed by an in-flight 508-wide
            # matmul.
            for lp in range(n_pairs - 2):
                _pair(lp, fine=(lp == 1))
            # Core 0's 30 rows stop at pair 14; ranks 1..7 also run pair 15.
            # Pair 15 is emitted BEFORE pair 14 so the last store in every
            # engine stream is unconditional (the scheduler otherwise parks
            # the post-If store behind the branch merge, adding ~500ns).
            with tc.If(pid > 0):
                _pair(n_pairs - 1)
            _pair(n_pairs - 2, tail=True)

    _split_multi_waits(nc)
    return nc


def _split_multi_waits(nc):
    """Walrus codegen accepts a single sync-wait command per instruction.

    Tile's sem assignment happily attaches several. Hoist all but the last
    wait of every instruction onto fresh NoOps placed immediately before it
    on the same engine stream (engine streams execute in program order, so
    semantics are preserved; the wait merely moves from the instruction to
    its dispatching sequencer).
    """
    # Program-order index of the last updater of each semaphore: multi-waits
    # are ordered so the latest-firing sem is checked last — earlier NoOps in
    # the hoisted chain are then already satisfied when reached.
    last_update = {}
    idx = 0
    for fn in nc.m.functions:
        for bb in fn.blocks:
            for inst in bb.instructions:
                si = inst.sync_info
                if si is not None and si.on_update:
                    for upd in si.on_update:
                        last_update[upd.id] = idx
                idx += 1
    for fn in nc.m.functions:
        for bb in fn.blocks:
            out = []
            for inst in bb.instructions:
                si = inst.sync_info
                waits = list(si.on_wait) if si is not None and si.on_wait else []
                if len(waits) > 1:
                    waits.sort(key=lambda w_: last_update.get(w_.id, -1))
                    for wt_ in waits[:-1]:
                        nop = mybir.InstNoOp(
                            name=nc.get_next_instruction_name(),
                            engine=inst.engine,
                        )
                        nop.sync_info = mybir.SyncInfo(
                            on_wait=[wt_], on_update=[]
                        )
                        nc.register_instruction(nop)
                        out.append(nop)
                    inst.sync_info = mybir.SyncInfo(
                        on_wait=[waits[-1]], on_update=list(si.on_update)
                    )
                out.append(inst)
            bb.instructions = out


def _get_program(*_args):
    key = ("v4", WARMUP_N)
    if key not in _COMPILED:
        _COMPILED[key] = _build_program(RPC)
    return _COMPILED[key]


def _core_rows(core):
    # Every core's tensors cover 32 rows; core 0 only computes/uses 30.
    h0 = 0 if core == 0 else RPC0 + RPC * (core - 1)
    return h0, RPC


def _prep_core(xp, core):
    bf16 = _np_bf16()
    h0, rows = _core_rows(core)
    xa = np.empty((128, rows, W), dtype=bf16)
    xa[:64] = xp[:, h0 : h0 + rows]
    xa[64:] = xp[:, h0 + 1 : h0 + 1 + rows]
    xb = np.zeros((128, rows, W), dtype=bf16)
    xb[:64] = xp[:, h0 + 2 : h0 + 2 + rows]
    xb[64:, :, : W - 1] = xp[:, h0 + 2 : h0 + 2 + rows, 1:]
    return xa.reshape(128, rows * W), xb.reshape(128, rows * W)


def _prep_wpack(kernels):
    bf16 = _np_bf16()
    # wpack[:, s, :]: s=kw in 0..2 -> (kh0 on partitions 0..63, kh1 on 64..127);
    # s=3 -> (kh2/kw0 on 0..63, kh2/kw1 on 64..127); s=4 -> (kh2/kw2 on 0..63).
    wpack = np.zeros((128, 5, 128), dtype=np.float32)
    for kw in range(KS):
        wpack[:64, kw, :] = kernels[:, :, 0, kw].T
        wpack[64:, kw, :] = kernels[:, :, 1, kw].T
    wpack[:64, 3, :] = kernels[:, :, 2, 0].T
    wpack[64:, 3, :] = kernels[:, :, 2, 1].T
    wpack[:64, 4, :] = kernels[:, :, 2, 2].T
    return wpack.reshape(128, 5 * 128).astype(bf16)


def _prep_inputs(x, kernels, biases, *_args):
    bf16 = _np_bf16()
    # Core 7 reads x rows up to 253 + 2 + 1 = 256: pad one row.
    xp = np.zeros((IN_C, H + 1, W), dtype=np.float32)
    xp[:, :H] = x
    xp = xp.astype(bf16)
    wpack = _prep_wpack(kernels)
    in_maps = []
    for core in range(N_CORES):
        xa, xb = _prep_core(xp, core)
        in_maps.append({"xa": xa, "xb": xb, "wpack": wpack})
    return in_maps


def kernel(x, kernels, biases):
    global LAST_RESULTS
    x = np.asarray(x, dtype=np.float32)
    kernels = np.asarray(kernels, dtype=np.float32)
    biases = np.asarray(biases, dtype=np.float32)

    nc = _get_program()
    in_maps = _prep_inputs(x, kernels, biases)
    res = run_bass_kernel_spmd(nc, in_maps, core_ids=list(range(N_CORES)), trace=TRACE)
    LAST_RESULTS = res

    out = np.empty((OUT_C, OH, OW), dtype=np.float32)
    for c in range(N_CORES):
        h0, _ = _core_rows(c)
        rows = RPC0 if c == 0 else RPC
        out[:, h0 : h0 + rows, :] = (
            res.results[c]["out"].astype(np.float32).reshape(OUT_C, RPC, OW)[:, :rows]
        )
    out = out + biases[:, None, None]
    return np.ascontiguousarray(out)


# revision 51
# speedup vs baseline: 1.0005x; 1.0005x over previous
"""Trainium2 Bass kernel for a 3x3 VALID conv: x[64,256,256] * k[128,64,3,3] -> [128,254,254].

Strategy:
  - Shard output rows across 8 cores: core 0 takes 30 rows, cores 1-7 take 32
    rows each (30 + 7*32 = 254, no padded rows anywhere). One SPMD program;
    the 16th row-pair is guarded by `partition_id > 0`.
  - bf16 operands (PE rate identical to f32r in the cost model; halves DMA bytes).
  - 5 accumulated matmuls per pair of output rows (the chain lower bound for a
    576-lane contraction at K<=128), using two SBUF x layouts:
      xa: partitions 0..63 = x rows q,   64..127 = x rows q+1  (row-shifted dup)
          -> covers taps (kh=0,kw)+(kh=1,kw) for kw=0,1,2      (3 matmuls)
      xb: partitions 0..63 = x rows q+2, 64..127 = x rows q+2 col-shifted +1
          -> covers taps (2,0)+(2,1) in one K=128 matmul       (1 matmul)
          -> tap (2,2) as a K=64 matmul on xb's lower half     (1 matmul)
  - PE p-state warm-up: dummy matmuls on a memset scratch tile keep the PE busy
    from ~0.7us so matmuls hit full clock as soon as the ramp window allows.
  - DMA queue split: xa + weights on the ACT HWDGE queue, xb on SP, stores on
    SP (queues transfer concurrently in the cost model; no engine-compute is
    placed on ACT/SP so DMAs never wedge behind it).
  - PSUM evacuation (fp32 psum -> bf16 SBUF) on DVE.
  - Bias is added on the host after the gather (biases are zeros here; the add
    is exact fp32 either way).
"""

import os
import sys

import numpy as np

for _p in ("/opt/trn_rl_repo", "/root/.axon_site/_ro/trn_rl_repo"):
    if os.path.isdir(_p) and _p not in sys.path:
        sys.path.insert(0, _p)

from concourse import bass, mybir, tile  # noqa: E402
from concourse.bass_utils import run_bass_kernel_spmd  # noqa: E402

IN_C, H, W = 64, 256, 256
KS = 3
OUT_C = 128
OH, OW = H - KS + 1, W - KS + 1  # 254, 254
N_CORES = 8
RPC0 = 30         # output rows on core 0
RPC = 32          # output rows on cores 1..7  (30 + 7*32 = 254)

# 12 x 32-wide M=1 warm-up matmuls bridge from the tiny DVE memset (first
# warm-up at ~0.40us) to the first w/xa DMAs landing (~0.70us locally;
# j0+j1's weights ride one bundled SP DMA), keeping the PE busy-streak
# unbroken so full clock engages at wall-clock ~3.0us. Undershooting the
# handoff (N<=11) costs ~850ns (ramp restart) — resweep after any change
# to the early DMA order.
WARMUP_N = int(os.environ.get("CONV_WARMUP_N", "12"))
WARMUP_W = int(os.environ.get("CONV_WARMUP_W", "32"))
WARMUP_M = int(os.environ.get("CONV_WARMUP_M", "0"))

# Matmul dtype (kept for test.py compatibility; the kernel is bf16).
MM_DT = "bf16"

TRACE = False
LAST_RESULTS = None

_COMPILED = {}


def _np_bf16():
    import ml_dtypes

    return np.dtype(ml_dtypes.bfloat16)


def _chunk_bounds(rows):
    # [2, 4, 4, ...] — a small first chunk lets pair 0 start as early as
    # possible; later chunks amortize per-DMA overhead.
    bounds = [0, 2]
    while bounds[-1] < rows:
        bounds.append(min(bounds[-1] + 4, rows))
    return bounds


def _build_program(rows):
    """One SPMD program for all 8 cores: 16 row-pairs, with the last pair
    guarded by `partition_id > 0` — core 0 (rank 0) computes only 15 pairs
    (30 rows), cores 1..7 compute all 16 (32 rows); 30 + 7*32 = 254."""
    bf16 = mybir.dt.bfloat16
    f32 = mybir.dt.float32
    n_pairs = rows // 2
    nc = bass.Bass()

    xa_ext = nc.declare_dram_parameter("xa", [128, rows * W], bf16, isOutput=False)
    xb_ext = nc.declare_dram_parameter("xb", [128, rows * W], bf16, isOutput=False)
    w_ext = nc.declare_dram_parameter("wpack", [128, 5 * 128], bf16, isOutput=False)
    o_ext = nc.declare_dram_parameter("out", [128, rows * OW], bf16, isOutput=True)

    with tile.TileContext(nc) as tc:
        with (
            tc.tile_pool(name="wpool", bufs=1) as wpool,
            tc.tile_pool(name="xpool", bufs=1) as xpool,
            tc.tile_pool(name="pwarm", bufs=1, space="PSUM") as pwarm,
            tc.tile_pool(name="pspool", bufs=7, space="PSUM") as pspool,
            # bufs = n pairs: output tiles are never reused -> evacuations
            # only ever wait on their PSUM producer.
            tc.tile_pool(name="opool", bufs=n_pairs + 1) as opool,
        ):
            # PE p-state warm-up (costs nothing: PE is idle while loads
            # land). M=1 warm-ups only need a [128, 1+W] scratch: the tiny
            # DVE memset finishes sooner, so the PE busy-streak starts
            # earlier and the full-clock point moves up with it.
            warm = wpool.tile([128, 1 + WARMUP_W], bf16)
            nc.vector.memset(warm[:], 0.0)
            pw = pwarm.tile([1, WARMUP_W], f32)
            for _ in range(WARMUP_N):
                nc.tensor.matmul(
                    pw[:],
                    lhsT=warm[:, 0:1],
                    rhs=warm[:, 1 : 1 + WARMUP_W],
                    start=True,
                    stop=True,
                )
            for _ in range(WARMUP_M):
                nc.tensor.matmul(
                    pw[:, 0:8],
                    lhsT=warm[:, 0:1],
                    rhs=warm[:, 1:9],
                    start=True,
                    stop=True,
                )

            wt = wpool.tile([128, 5 * 128], bf16)
            # First DMAs on both queues land at ~700ns regardless of size
            # (fixed DGE costs dominate). Bundle w slots 0+1 into SP's first
            # DMA so j0 AND j1 are unblocked at ~800ns; w slots 2-4 follow
            # xa0 on ACT and land before j2 needs them.
            nc.sync.dma_start(out=wt[:, 0:256], in_=w_ext[:, 0:256])

            xat = xpool.tile([128, rows * W], bf16)
            xbt = xpool.tile([128, rows * W], bf16)
            bounds = _chunk_bounds(rows)
            for i, (q0, q1) in enumerate(zip(bounds[:-1], bounds[1:])):
                # First xa chunk rides the ACT queue in parallel with w0 on
                # SP (j0 needs both); everything else: xa on ACT, xb on SP.
                nc.scalar.dma_start(
                    out=xat[:, q0 * W : q1 * W], in_=xa_ext[:, q0 * W : q1 * W]
                )
                if i == 0:
                    nc.scalar.dma_start(out=wt[:, 256:], in_=w_ext[:, 256:])
                nc.sync.dma_start(
                    out=xbt[:, q0 * W : q1 * W], in_=xb_ext[:, q0 * W : q1 * W]
                )

            # Dummy activation AFTER the load dispatches: loads the ACT
            # function table off the critical path so the final pair's
            # evacuation can use the ACT engine.
            actwarm = wpool.tile([128, 1], bf16)
            nc.scalar.copy(actwarm[:], warm[:, 0:1])

            wv = wt[:].rearrange("p (s m) -> p s m", m=128)
            ov = o_ext.rearrange("p (r w) -> p r w", w=OW)
            xav = xat[:].rearrange("p (q w) -> p q w", w=W)
            xbv = xbt[:].rearrange("p (q w) -> p q w", w=W)

            pid = nc.partition_id()

            def _rows(r, nr, ps, evac, fine=False):
                """One PSUM accumulation group covering output rows r..r+nr.

                fine=True splits every matmul into quarter-size pieces: the
                p-state (mid vs full clock) is sampled at instruction issue,
                so fine pieces let the rate flip mid-group where a 508-wide
                matmul straddling the 3us full-clock point would pay mid
                rate for its whole duration.
                """
                psv = ps.rearrange("p (r w) -> p r w", w=OW)
                HW = OW // 2  # 127

                def mm(lhsT, rhs_view, base_c, start, stop):
                    if not fine:
                        nc.tensor.matmul(
                            psv[:, 0:nr, :],
                            lhsT=lhsT,
                            rhs=rhs_view[:, r : r + nr, base_c : base_c + OW],
                            start=start,
                            stop=stop,
                        )
                        return
                    QW = OW // 4  # 63
                    cols = [0, QW, 2 * QW, 3 * QW]
                    n_piece = len(cols) * nr
                    pi = 0
                    for i in range(nr):
                        for c0 in cols:
                            cw = OW - c0 if c0 == 3 * QW else QW
                            nc.tensor.matmul(
                                psv[:, i : i + 1, c0 : c0 + cw],
                                lhsT=lhsT,
                                rhs=rhs_view[
                                    :, r + i : r + i + 1,
                                    base_c + c0 : base_c + c0 + cw,
                                ],
                                start=start and pi == 0,
                                stop=stop and pi == n_piece - 1,
                            )
                            pi += 1

                for kw in range(3):
                    mm(wv[:, kw, :], xav, kw, kw == 0, False)
                mm(wv[:, 3, :], xbv, 0, False, False)
                mm(wv[0:64, 4, :], xbv[0:64], 2, False, True)
                so = opool.tile([128, nr * OW], bf16, name="so")
                evac(so[:], ps[:])
                nc.sync.dma_start(out=ov[:, r : r + nr, :], in_=so[:])

            def _pair(lp, tail=False, fine=False):
                r = 2 * lp
                ps = pspool.tile([128, 2 * OW], f32, name="ps")
                if tail:
                    # Exit critical path: ACT evac is slightly faster than DVE
                    # (table preloaded above).
                    _rows(r, 2, ps, lambda o, p: nc.scalar.copy(o, p))
                else:
                    _rows(r, 2, ps,
                          lambda o, p: nc.vector.tensor_scalar_add(o, p, 0.0),
                          fine=fine)

            # Pair 1 straddles the wall-clock 3us full-speed point: emit it
            # in quarter-size pieces so the clock-rate flip (sampled at
            # instruction issue) isn't delayed by an in-flight 508-wide
            # matmul.
            for lp in range(n_pairs - 2):
                _pair(lp, fine=(lp == 1))
            # Core 0's 30 rows stop at pair 14; ranks 1..7 also run pair 15.
            # Pair 15 is emitted BEFORE pair 14 so the last store in every
            # engine stream is unconditional (the scheduler otherwise parks
            # the post-If store behind the branch merge, adding ~500ns).
            with tc.If(pid > 0):
                _pair(n_pairs - 1)
            _pair(n_pairs - 2, tail=True)

    _split_multi_waits(nc)
    return nc


def _split_multi_waits(nc):
    """Walrus codegen accepts a single sync-wait command per instruction.

    Tile's sem assignment happily attaches several. Hoist all but the last
    wait of every instruction onto fresh NoOps placed immediately before it
    on the same engine stream (engine streams execute in program order, so
    semantics are preserved; the wait merely moves from the instruction to
    its dispatching sequencer).
    """
    # Program-order index of the last updater of each semaphore: multi-waits
    # are ordered so the latest-firing sem is checked last — earlier NoOps in
    # the hoisted chain are then already satisfied when reached.
    last_update = {}
    idx = 0
    for fn in nc.m.functions:
        for bb in fn.blocks:
            for inst in bb.instructions:
                si = inst.sync_info
                if si is not None and si.on_update:
                    for upd in si.on_update:
                        last_update[upd.id] = idx
                idx += 1
    for fn in nc.m.functions:
        for bb in fn.blocks:
            out = []
            for inst in bb.instructions:
                si = inst.sync_info
                waits = list(si.on_wait) if si is not None and si.on_wait else []
                if len(waits) > 1:
                    waits.sort(key=lambda w_: last_update.get(w_.id, -1))
                    for wt_ in waits[:-1]:
                        nop = mybir.InstNoOp(
                            name=nc.get_next_instruction_name(),
                            engine=inst.engine,
                        )
                        nop.sync_info = mybir.SyncInfo(
                            on_wait=[wt_], on_update=[]
                        )
                        nc.register_instruction(nop)
                        out.append(nop)
                    inst.sync_info = mybir.SyncInfo(
                        on_wait=[waits[-1]], on_update=list(si.on_update)
                    )
                out.append(inst)
            bb.instructions = out


def _get_program(*_args):
    key = ("v4", WARMUP_N)
    if key not in _COMPILED:
        _COMPILED[key] = _build_program(RPC)
    return _COMPILED[key]


def _core_rows(core):
    # Every core's tensors cover 32 rows; core 0 only computes/uses 30.
    h0 = 0 if core == 0 else RPC0 + RPC * (core - 1)
    return h0, RPC


def _prep_core(xp, core):
    bf16 = _np_bf16()
    h0, rows = _core_rows(core)
    xa = np.empty((128, rows, W), dtype=bf16)
    xa[:64] = xp[:, h0 : h0 + rows]
    xa[64:] = xp[:, h0 + 1 : h0 + 1 + rows]
    xb = np.zeros((128, rows, W), dtype=bf16)
    xb[:64] = xp[:, h0 + 2 : h0 + 2 + rows]
    xb[64:, :, : W - 1] = xp[:, h0 + 2 : h0 + 2 + rows, 1:]
    return xa.reshape(128, rows * W), xb.reshape(128, rows * W)


def _prep_wpack(kernels):
    bf16 = _np_bf16()
    # wpack[:, s, :]: s=kw in 0..2 -> (kh0 on partitions 0..63, kh1 on 64..127);
    # s=3 -> (kh2/kw0 on 0..63, kh2/kw1 on 64..127); s=4 -> (kh2/kw2 on 0..63).
    wpack = np.zeros((128, 5, 128), dtype=np.float32)
    for kw in range(KS):
        wpack[:64, kw, :] = kernels[:, :, 0, kw].T
        wpack[64:, kw, :] = kernels[:, :, 1, kw].T
    wpack[:64, 3, :] = kernels[:, :, 2, 0].T
    wpack[64:, 3, :] = kernels[:, :, 2, 1].T
    wpack[:64, 4, :] = kernels[:, :, 2, 2].T
    return wpack.reshape(128, 5 * 128).astype(bf16)


def _prep_inputs(x, kernels, biases, *_args):
    bf16 = _np_bf16()
    # Core 7 reads x rows up to 253 + 2 + 1 = 256: pad one row.
    xp = np.zeros((IN_C, H + 1, W), dtype=np.float32)
    xp[:, :H] = x
    xp = xp.astype(bf16)
    wpack = _prep_wpack(kernels)
    in_maps = []
    for core in range(N_CORES):
        xa, xb = _prep_core(xp, core)
        in_maps.append({"xa": xa, "xb": xb, "wpack": wpack})
    return in_maps


def kernel(x, kernels, biases):
    global LAST_RESULTS
    x = np.asarray(x, dtype=np.float32)
    kernels = np.asarray(kernels, dtype=np.float32)
    biases = np.asarray(biases, dtype=np.float32)

    nc = _get_program()
    in_maps = _prep_inputs(x, kernels, biases)
    res = run_bass_kernel_spmd(nc, in_maps, core_ids=list(range(N_CORES)), trace=TRACE)
    LAST_RESULTS = res

    out = np.empty((OUT_C, OH, OW), dtype=np.float32)
    for c in range(N_CORES):
        h0, _ = _core_rows(c)
        rows = RPC0 if c == 0 else RPC
        out[:, h0 : h0 + rows, :] = (
            res.results[c]["out"].astype(np.float32).reshape(OUT_C, RPC, OW)[:, :rows]
        )
    out = out + biases[:, None, None]
    return np.ascontiguousarray(out)
